# revision 1
# baseline (speedup 1.0000x reference)
"""Trainium2 Bass kernel for the dense branch-MLP problem.

Computes: out[b,o] = sum_n relu((s[b,:] - v[n,:]) @ W[n].T + bias[n])[o]
with B=1024, N=64, D=512, OUT=2048 in fp32.

Sharding: expert-style across the N=64 branch axis -> 8 branches per core.
Each core computes a full [B, OUT] partial sum over its 8 branches; the
host sums the 8 partials (the unshard step).

Per-core kernel (PE-bound, ~17.2 GFLOP at fp32r rates):
  - s^T resident in SBUF as 4 d-chunks [128, 1024]
  - per branch: offs = s^T - v_n (VectorE tensor_scalar, per-partition
    scalar), stream W[n]^T tiles as matmul stationary operands, accumulate
    over the 4 d-chunks in PSUM (8 interleaved bank groups so PE starts as
    soon as the first weight/offset chunks land), relu+bias on ScalarE,
    branch-sum on VectorE, per-(o,b)-tile output DMA.
  - matmuls run in float32r (fp22 internal) at 1 cycle/row since the
    moving free dim is 512 -> full bf16-class PE throughput with ~1e-4
    relative accuracy vs the fp32 reference.
  - a tiny-matmul warmup burst during the startup DMA window brings the
    PE HAM clock gate to 8/8 (2.4 GHz) before the first real matmul.

Cost-model timeline: ~235.6 us/core (PE busy ~221 us = 94%, vs a 218.5 us
theoretical floor for 1024 N=512 matmuls); validated on hardware
early-session at ~+3% (269.7 us measured vs 261.4 us predicted for the
baseline version of this kernel). Critical ordering detail: the bias DMA
loads FIRST — it gates the batch-0 relu drain and, through PSUM slot
recycling, every later matmul batch.
"""

import numpy as np

import concourse.bacc as bacc
import concourse.mybir as mybir
import concourse.tile as tile
from concourse.bass_utils import run_bass_kernel_spmd

B, N, D, OUT = 1024, 64, 512, 2048
N_CORES = 8
NL = N // N_CORES  # branches per core
DC = D // 128  # d chunks (4)
OT = OUT // 128  # o tiles (16)
BT = B // 512  # b free-dim tiles (2)

F32 = mybir.dt.float32
F32R = mybir.dt.float32r
BF16 = mybir.dt.bfloat16
RELU = mybir.ActivationFunctionType.Relu
IDENT = mybir.ActivationFunctionType.Identity

_cache = {}


def build(repeat: int = 1):
    """Build + compile the per-core Bass program. Cached per `repeat`."""
    if repeat in _cache:
        return _cache[repeat]

    nc = bacc.Bacc(
        "TRN2",
        target_bir_lowering=False,
        debug=False,
        num_devices=N_CORES,
    )

    wt_d = nc.dram_tensor("wt", [NL, 128, DC * OUT], F32R, kind="ExternalInput").ap()
    st_d = nc.dram_tensor("st", [128, DC * B], F32, kind="ExternalInput").ap()
    negv_d = nc.dram_tensor("negv", [128, NL * DC], F32, kind="ExternalInput").ap()
    bias_d = nc.dram_tensor("bias", [128, NL * OT], F32, kind="ExternalInput").ap()
    out_d = nc.dram_tensor("out", [OUT, B], F32, kind="ExternalOutput").ap()

    # o-range chunks per weight DMA: each chunk delivers o_tiles for all DC
    # d-chunks so matmul groups become ready progressively.
    WCH = 8  # wt DMA chunks per branch
    OT_PER_CH = OT // WCH

    with tile.TileContext(nc) as tc:
        with (
            tc.tile_pool(name="const", bufs=1) as const_pool,
            tc.tile_pool(name="acc", bufs=1) as acc_pool,
            tc.tile_pool(name="offs", bufs=2) as offs_pool,
            tc.tile_pool(name="wt", bufs=2) as wt_pool,
            tc.tile_pool(name="tmp", bufs=6) as tmp_pool,
            tc.tile_pool(name="psum", bufs=8, space="PSUM") as psum_pool,
        ):
            def wt_chunk_dma(wt, n, j, nch=WCH):
                wt3 = wt[:].rearrange("p (c o) -> p c o", c=DC)
                wd3 = wt_d[n].rearrange("p (c o) -> p c o", c=DC)
                osz = (OT // nch) * 128
                nc.sync.dma_start(
                    wt3[:, :, j * osz : (j + 1) * osz],
                    wd3[:, :, j * osz : (j + 1) * osz],
                )

            # Startup order matters: the first batch's c-outer matmuls need
            # ALL FOUR offs chunks (hence all of st) within ~7us of the first
            # matmul, while weight chunks are consumed at only ~1.7us each.
            # So: st0 + the first weight chunk to start PE, then the REST of
            # st immediately (offsets pace the first batch), then the
            # remaining branch-0 weight chunks.
            negv = const_pool.tile([128, NL * DC], F32, name="negv")
            nc.sync.dma_start(negv[:], negv_d[:])
            # bias is tiny but gates the batch-0 relu drain (and through PSUM
            # slot recycling, every later batch) -> load it FIRST.
            bias = const_pool.tile([128, NL * OT], F32, name="bias")
            nc.sync.dma_start(bias[:], bias_d[:])
            st = const_pool.tile([128, DC * B], F32, name="st")
            wt0 = wt_pool.tile([128, DC * OUT], F32R, name="wt_t", tag="wt_t")
            nc.sync.dma_start(st[:, 0:B], st_d[:, 0:B])
            wt_chunk_dma(wt0, 0, 0)
            wt_chunk_dma(wt0, 0, 1)
            for c in range(1, DC):
                nc.sync.dma_start(
                    st[:, c * B : (c + 1) * B], st_d[:, c * B : (c + 1) * B]
                )
            for j in range(2, WCH):
                wt_chunk_dma(wt0, 0, j)

            acc = [
                acc_pool.tile([128, B], F32, name=f"acc{ot}", tag=f"acc{ot}")
                for ot in range(OT)
            ]

            # PE warmup: a burst of tiny matmuls on scratch data during the
            # startup DMA window, so the HAM clock gate reaches 8/8 (2.4 GHz)
            # before the first real matmul issues.
            scr = const_pool.tile([128, 128], BF16, name="scr")
            nc.vector.memset(scr[:], 0.0)
            wps = psum_pool.tile([128, 512], F32, name="wps", tag="ps")
            for _ in range(56):
                nc.tensor.matmul(
                    wps[0:64, 0:64], scr[:, 0:64], scr[:, 64:128], start=True, stop=True
                )

            def load_wt(n):
                wt = wt_pool.tile([128, DC * OUT], F32R, name="wt_t", tag="wt_t")
                for j in range(WCH):
                    wt_chunk_dma(wt, n, j)
                return wt

            def make_offs(n, dt=F32R):
                offs = offs_pool.tile([128, DC * B], dt, name="offs", tag="offs")
                for c in range(DC):
                    nc.vector.tensor_scalar_add(
                        offs[:, c * B : (c + 1) * B],
                        st[:, c * B : (c + 1) * B],
                        negv[:, n * DC + c : n * DC + c + 1],
                    )
                return offs

            groups = [(ot, bt) for ot in range(OT) for bt in range(BT)]
            BATCH = 8  # interleaved psum groups (= psum banks)

            def drain_group(n, ps, ot, bt):
                b_ap = bias[:, n * OT + ot : n * OT + ot + 1]
                if n == 0:
                    nc.scalar.activation(
                        acc[ot][:, bt * 512 : bt * 512 + 512],
                        ps[:],
                        RELU,
                        bias=b_ap,
                        scale=1.0,
                    )
                else:
                    tmp = tmp_pool.tile([128, 512], F32, name="tmp", tag="tmp")
                    nc.scalar.activation(tmp[:], ps[:], RELU, bias=b_ap, scale=1.0)
                    nc.vector.tensor_add(
                        acc[ot][:, bt * 512 : bt * 512 + 512],
                        acc[ot][:, bt * 512 : bt * 512 + 512],
                        tmp[:],
                    )
                if n == NL - 1:
                    nc.sync.dma_start(
                        out_d[ot * 128 : (ot + 1) * 128, bt * 512 : bt * 512 + 512],
                        acc[ot][:, bt * 512 : bt * 512 + 512],
                    )

            def body(iv=None):
                for n in range(NL):
                    wt = wt0 if n == 0 else load_wt(n)
                    offs = make_offs(n)

                    last_branch = n == NL - 1
                    for g0 in range(0, len(groups), BATCH):
                        batch = groups[g0 : g0 + BATCH]
                        last_batch = last_branch
                        pss = [
                            psum_pool.tile([128, 512], F32, name="ps", tag="ps")
                            for _ in batch
                        ]
                        if last_batch:
                            # c-inner: groups finish one at a time so the
                            # ACT/DVE/DMA drain trickles instead of bunching
                            # after the final matmul.
                            for ps, (ot, bt) in zip(pss, batch):
                                for c in range(DC):
                                    nc.tensor.matmul(
                                        ps[:],
                                        wt[:, c * OUT + ot * 128 : c * OUT + (ot + 1) * 128],
                                        offs[:, c * B + bt * 512 : c * B + bt * 512 + 512],
                                        start=(c == 0),
                                        stop=(c == DC - 1),
                                    )
                                drain_group(n, ps, ot, bt)
                        else:
                            # d-chunk outer, group inner: PE starts as soon as
                            # the first offs/wt chunks land; later chunks
                            # stream in behind.
                            for c in range(DC):
                                for ps, (ot, bt) in zip(pss, batch):
                                    nc.tensor.matmul(
                                        ps[:],
                                        wt[:, c * OUT + ot * 128 : c * OUT + (ot + 1) * 128],
                                        offs[:, c * B + bt * 512 : c * B + bt * 512 + 512],
                                        start=(c == 0),
                                        stop=(c == DC - 1),
                                    )
                            for ps, (ot, bt) in zip(pss, batch):
                                drain_group(n, ps, ot, bt)

            if repeat == 1:
                body()
            else:
                with tc.For_i(0, repeat, 1):
                    body()

    nc.compile()
    _cache[repeat] = nc
    return nc


def prep_inputs(semantic_vec, vertices, W, b):
    """Host-side layout transforms -> per-core input maps."""
    semantic_vec = np.asarray(semantic_vec, dtype=np.float32)
    vertices = np.asarray(vertices, dtype=np.float32)
    W = np.asarray(W, dtype=np.float32)
    b = np.asarray(b, dtype=np.float32)

    # st[p, c*B + bb] = s[bb, c*128+p]
    st = np.ascontiguousarray(
        semantic_vec.reshape(B, DC, 128).transpose(2, 1, 0).reshape(128, DC * B)
    )
    # wt[n, p, c*OUT + o] = W[n, o, c*128+p]
    wt = np.ascontiguousarray(
        W.reshape(N, OUT, DC, 128).transpose(0, 3, 2, 1).reshape(N, 128, DC * OUT)
    )
    # negv[p, nl*DC + c] = -v[n0+nl, c*128+p]
    negv = np.ascontiguousarray(
        (-vertices).reshape(N_CORES, NL, DC, 128).transpose(0, 3, 1, 2).reshape(N_CORES, 128, NL * DC)
    )
    # bias[p, nl*OT + ot] = b[n0+nl, ot*128+p]
    bias = np.ascontiguousarray(
        b.reshape(N_CORES, NL, OT, 128).transpose(0, 3, 1, 2).reshape(N_CORES, 128, NL * OT)
    )

    in_maps = []
    for core in range(N_CORES):
        in_maps.append(
            {
                "wt": wt[core * NL : (core + 1) * NL],
                "st": st,
                "negv": negv[core],
                "bias": bias[core],
            }
        )
    return in_maps


def kernel(semantic_vec, vertices, W, b):
    nc = build(repeat=1)
    in_maps = prep_inputs(semantic_vec, vertices, W, b)
    res = run_bass_kernel_spmd(nc, in_maps, core_ids=list(range(N_CORES)))
    total = np.zeros((OUT, B), dtype=np.float32)
    for core in range(N_CORES):
        total += res.results[core]["out"]
    return np.ascontiguousarray(total.T)



# revision 2
# speedup vs baseline: 2.0671x; 2.0671x over previous
"""Trainium2 Bass kernel for the dense branch-MLP problem (fp8 DoubleRow).

Computes: out[b,o] = sum_n relu((s[b,:] - v[n,:]) @ W[n].T + bias[n])[o]
with B=1024, N=64, D=512, OUT=2048 in fp32.

Sharding: expert-style across the N=64 branch axis -> 8 branches per core.
Each core computes a full [B, OUT] partial sum over its 8 branches; the
host sums the 8 partials (the unshard step).

Key reformulation: the vertex offset folds into a per-(n,o) constant
  c[n,o] = bias[n,o] - v[n] @ W[n].T        (host, float64)
so the device work per branch is
  acc[o,b] += relu((s @ (16 W[n]).T)[b,o] + 16 c[n,o])
with s and 16*W quantized host-side to fp8-e4m3 and the matmuls run in
DoubleRow perf mode (2 fp8 MACs per PE cell per cycle, effective contract
256 per matmul -> 2 matmuls instead of 4 per [128o x 512b] group, each at
0.5 cycles/output-row). The x16 scaling keeps W out of the e4m3 subnormal
range (unscaled, 35% of |W|<=0.044 values fall below the 2^-6 normal
floor); the matching /16 happens on the host after the partial-sum gather.
Numerically validated vs the f64 reference: rel-absmax err ~1.3e-2
(gate 2e-2); the fp16 on-chip accumulate adds <1e-4.

Per-core engine budget (cost model rates):
  PE : 8 branches x 16 o-tiles x 4 DoubleRow matmuls (2 c-halves x 2
       b-halves) = 512 matmuls x ~107ns = ~55us (sim; ~2x that on HW).
  R-pass (relu+bias, PSUM fp32 -> SBUF fp16, 16.8M elems): split
       ACT (activation, bias fused) ~70% / DVE (tensor_scalar add,max) ~30%.
  A-pass (acc += tmp, fp16, eligible for DVE 2x_1p): split
       DVE tensor_add ~62% / gpsimd tensor_add ~38% (Add eff 0.42).
  All weights SBUF-resident (64KB/partition); no per-branch offset
  compute (folded into c), so DVE has no other work.
"""

import numpy as np

import concourse.bacc as bacc
import concourse.mybir as mybir
import concourse.tile as tile
from concourse.bass_utils import run_bass_kernel_spmd

B, N, D, OUT = 1024, 64, 512, 2048
N_CORES = 8
NL = N // N_CORES  # branches per core (8)
OT = OUT // 128    # o tiles (16)
C2 = 2             # contract super-chunks of 256 (DoubleRow)
J = 2              # fp8 pair dim per super-chunk
BT = 2             # b halves of 512 (one PSUM bank each)
CJO = C2 * J * OUT # flat wt free size per branch (8192)

F32 = mybir.dt.float32
F16 = mybir.dt.float16
F8 = mybir.dt.float8e4
BF16 = mybir.dt.bfloat16
RELU = mybir.ActivationFunctionType.Relu
DR = mybir.MatmulPerfMode.DoubleRow
ADD = mybir.AluOpType.add
MAX = mybir.AluOpType.max

# Engine-split weights for the elementwise passes (fractions routed to the
# first-listed engine); tuned against the cost model.
R_ACT_W = 0.70   # relu pass: ACT share (rest DVE)
A_DVE_W = 0.625  # accumulate pass: DVE share (rest gpsimd)

_cache = {}


def build(repeat: int = 1):
    """Build + compile the per-core Bass program. Cached per `repeat`."""
    if repeat in _cache:
        return _cache[repeat]

    nc = bacc.Bacc(
        "TRN2",
        target_bir_lowering=False,
        debug=False,
        num_devices=N_CORES,
    )

    wt_d = nc.dram_tensor("wt", [NL, 128, CJO], F8, kind="ExternalInput").ap()
    st_d = nc.dram_tensor("st", [128, C2 * J * B], F8, kind="ExternalInput").ap()
    c16_d = nc.dram_tensor("c16", [128, NL * OT], F32, kind="ExternalInput").ap()
    out_d = nc.dram_tensor("out", [OUT, B], F16, kind="ExternalOutput").ap()

    with tile.TileContext(nc) as tc:
        with (
            tc.tile_pool(name="const", bufs=1) as const_pool,
            tc.tile_pool(name="acc", bufs=1) as acc_pool,
            tc.tile_pool(name="tmp", bufs=8) as tmp_pool,
            tc.tile_pool(name="psum", bufs=4, space="PSUM") as psum_pool,
        ):
            # c16 is tiny but gates every R-op -> load FIRST.
            c16 = const_pool.tile([128, NL * OT], F32, name="c16")
            nc.sync.dma_start(c16[:], c16_d[:])
            # s (fp8, 4KB/partition) gates all matmuls.
            st = const_pool.tile([128, C2 * J * B], F8, name="st")
            nc.sync.dma_start(st[:], st_d[:])
            # All 8 branches' weights are SBUF-resident (8KB/partition each).
            # Branch 0 is split by o-range so the first o-tiles unlock early.
            wt = const_pool.tile([128, NL * CJO], F8, name="wt")
            wt5 = wt[:].rearrange("p (n c j o) -> p n c j o", n=NL, c=C2, j=J)
            wd4 = [wt_d[n].rearrange("p (c j o) -> p c j o", c=C2, j=J) for n in range(NL)]
            nc.sync.dma_start(wt5[:, 0, :, :, 0:1024], wd4[0][:, :, :, 0:1024])
            nc.sync.dma_start(wt5[:, 0, :, :, 1024:2048], wd4[0][:, :, :, 1024:2048])
            for n in range(1, NL):
                nc.sync.dma_start(wt[:, n * CJO : (n + 1) * CJO], wt_d[n])

            st4 = st[:].rearrange("p (c j b) -> p c j b", c=C2, j=J)

            acc = [
                acc_pool.tile([128, B], F16, name=f"acc{ot}", tag=f"acc{ot}")
                for ot in range(OT)
            ]

            # PE warmup: a burst of tiny matmuls on scratch data during the
            # startup DMA window, so the HAM clock gate reaches 8/8 (2.4 GHz)
            # before the first real matmul issues.
            scr = const_pool.tile([128, 128], BF16, name="scr")
            nc.vector.memset(scr[:], 0.0)
            wps = psum_pool.tile([128, B], F32, name="wps", tag="ps")
            for _ in range(56):
                nc.tensor.matmul(
                    wps[0:64, 0:64], scr[:, 0:64], scr[:, 64:128], start=True, stop=True
                )

            def body(iv=None):
                r_bal = 0.0
                a_bal = 0.0
                for n in range(NL):
                    for ot in range(OT):
                        ps = psum_pool.tile([128, B], F32, name="ps", tag="ps")
                        for c2 in range(C2):
                            lhsT = wt5[:, n, c2, :, ot * 128 : (ot + 1) * 128]
                            for bt in range(BT):
                                nc.tensor.matmul(
                                    ps[:, bt * 512 : (bt + 1) * 512],
                                    lhsT,
                                    st4[:, c2, :, bt * 512 : (bt + 1) * 512],
                                    start=(c2 == 0),
                                    stop=(c2 == C2 - 1),
                                    perf_mode=DR,
                                )
                        b_ap = c16[:, n * OT + ot : n * OT + ot + 1]
                        if n == 0:
                            dst = acc[ot][:]
                        else:
                            dst = tmp_pool.tile([128, B], F16, name="tmp", tag="tmp")[:]
                        r_bal += R_ACT_W
                        if r_bal >= 1.0:
                            r_bal -= 1.0
                            nc.scalar.activation(dst, ps[:], RELU, bias=b_ap, scale=1.0)
                        else:
                            nc.vector.tensor_scalar(dst, ps[:], b_ap, 0.0, ADD, MAX)
                        if n > 0:
                            a_bal += A_DVE_W
                            if a_bal >= 1.0:
                                a_bal -= 1.0
                                nc.vector.tensor_add(acc[ot][:], acc[ot][:], dst)
                            else:
                                nc.gpsimd.tensor_add(acc[ot][:], acc[ot][:], dst)
                        if n == NL - 1:
                            nc.sync.dma_start(
                                out_d[ot * 128 : (ot + 1) * 128, :], acc[ot][:]
                            )

            if repeat == 1:
                body()
            else:
                with tc.For_i(0, repeat, 1):
                    body()

    nc.compile()
    _cache[repeat] = nc
    return nc


def prep_inputs(semantic_vec, vertices, W, b):
    """Host-side quantization + layout transforms -> per-core input maps."""
    s = np.asarray(semantic_vec, dtype=np.float32)
    v = np.asarray(vertices, dtype=np.float32)
    W = np.asarray(W, dtype=np.float32)
    bb = np.asarray(b, dtype=np.float32)
    f8 = mybir.dt.np(F8)

    # st[p, c2*J*B + j*B + bb] = s[bb, c2*256 + j*128 + p]
    st = np.ascontiguousarray(
        s.reshape(B, C2, J, 128).transpose(3, 1, 2, 0).reshape(128, C2 * J * B)
    ).astype(f8)
    # wt[n][p, c2, j, o] = 16 * W[n, o, c2*256 + j*128 + p]
    wt = np.ascontiguousarray(
        (W * np.float32(16.0))
        .reshape(N, OUT, C2, J, 128)
        .transpose(0, 4, 2, 3, 1)
        .reshape(N, 128, CJO)
    ).astype(f8)
    # c16[core][p, nl*OT + ot] = 16 * (b[n] - v[n] @ W[n].T)[ot*128 + p]
    c = np.empty((N, OUT), dtype=np.float64)
    v64 = v.astype(np.float64)
    for n in range(N):
        c[n] = bb[n].astype(np.float64) - W[n].astype(np.float64) @ v64[n]
    c16 = np.ascontiguousarray(
        (16.0 * c)
        .astype(np.float32)
        .reshape(N_CORES, NL, OT, 128)
        .transpose(0, 3, 1, 2)
        .reshape(N_CORES, 128, NL * OT)
    )

    in_maps = []
    for core in range(N_CORES):
        in_maps.append(
            {
                "wt": wt[core * NL : (core + 1) * NL],
                "st": st,
                "c16": c16[core],
            }
        )
    return in_maps


def kernel(semantic_vec, vertices, W, b):
    nc = build(repeat=1)
    in_maps = prep_inputs(semantic_vec, vertices, W, b)
    res = run_bass_kernel_spmd(nc, in_maps, core_ids=list(range(N_CORES)))
    total = np.zeros((OUT, B), dtype=np.float32)
    for core in range(N_CORES):
        total += np.asarray(res.results[core]["out"]).astype(np.float32)
    total *= 1.0 / 16.0
    return np.ascontiguousarray(total.T)
